# revision 22
# baseline (speedup 1.0000x reference)
"""CLIP-style contrastive (HCL) loss for B=4096, f32 logits on 8 trn2 cores.

Math reduction (BETA=1, t=0.5, tau+=0.1):
  - imp == neg, so reweight_neg = sum(neg^2) * N / sum(neg).
  - Row i and row i+B of the 2Bx2B sim matrix hold identical value multisets,
    so loss[i] == loss[i+B] and the mean over 2B rows == mean over B rows.
  - Everything reduces to row sums + col sums of E = exp(2L) and E2 = exp(4L),
    plus the diagonal of L.

Device work per core (rows k*512..(k+1)*512 of L, cast to bf16 on host):
  - 4 row-tiles [128, 4096]. ACT: exp(2x)->bf16 E with fused fp32 row-sum,
    pieces <=2048 wide so DVE can chase piece-by-piece (4096-wide pieces
    starve DVE: its chase granularity is the producing instruction). Tile 0
    leads with 2x1024 pieces so both ACT and DVE ramp early.
  - DVE: E2 = E*E via scalar_tensor_tensor (1x, ~1.04 ns/col) with fused
    fp32 row-sum, one op per ACT piece, skipping the trailing E4 span.
  - Engine balance: DVE (1.04/col) > ACT (0.833/col), so the trailing
    E4_W columns of tile 3 get E2 = exp(4x) directly on ACT (fused row-sum)
    instead of a DVE square. E4_W tunes ACT-vs-DVE co-termination.
  - PE: ones-matmul per 128-col block accumulates per-column sums of E and
    E2 into PSUM (chunk-stationary layout [128, 32] each).
  - Input DMAs on the sync HWDGE queue, one per ACT piece (gpsimd DMAs go
    through SWDGE = slow; whole-tile DMAs delay the first piece of a tile).
    Output DMAs spread over the sync + scalar HWDGE queues so their ~0.6us
    issue costs overlap; PSUM evictions both on the (idle by then) DVE.
Host: assemble sums, per-row loss formula over 4096 rows in f64, mean.

Measured structure (trn2, per-core): ~4.7us fixed ramp (barrier -> first
DMA ready incl ~2us DMA-completion semaphore propagation), ~19.3us balanced
ACT/DVE compute (exp floor 13.7us + ~0.28us/op ACT init), ~3us PSUM-evict/
DMA-out chain, ~8.4us fixed NEFF exit (full semaphore-file reset, paced by
the PE queue at ~130ns/clear). Engines that cannot help: Pool/GpSimd
scalar_tensor_tensor is rejected by the ISA check and Pool tensor_tensor
wedges the device (unimplemented Q7 op); DVE 2x-perf-mode ops (tensor_tensor
or tensor_mask_reduce) total the same ns/col as the fused 1x STT.
"""

import os

import numpy as np
import ml_dtypes

import concourse.bacc as bacc
import concourse.bass as bass
import concourse.tile as tile
from concourse import mybir
from concourse.bass_utils import run_bass_kernel_spmd

B = 4096
N_CORES = 8
ROWS_PER_CORE = B // N_CORES  # 512
P = 128
TILES = ROWS_PER_CORE // P  # 4

TAU_PLUS = 0.1
TEMPERATURE = 0.5
EPS = 1e-8

# exp(2x) piece widths per tile (ACT ops; also the DVE chase granularity).
_def_pieces = "1024,1024,2048;2048,2048;2048,2048;2048,2048"
PIECE_CFG = os.environ.get("KERNEL_PIECES", _def_pieces)
ACT_PIECES = [
    [int(x) for x in part.split(",")] for part in PIECE_CFG.split(";")
]
assert len(ACT_PIECES) == TILES and all(sum(p) == B for p in ACT_PIECES)
# Trailing columns of tile 3 whose E2 comes from ACT exp(4x) instead of a
# DVE square (engine balancing). Multiple of 128.
E4_W = int(os.environ.get("KERNEL_E4_W", "1536"))
assert E4_W % P == 0 and 0 < E4_W < B  # >0: the psE2 stop lands on the e4 blocks
E4_START = B - E4_W
# DMA granularity: per exp2 piece (1) or per tile after tile 0 (0).
DMA_SPLIT_LATER = bool(int(os.environ.get("KERNEL_DMA_SPLIT", "1")))
# DMA issue queue. gpsimd routes through SWDGE (software descriptors, slow);
# sync/scalar have hardware DGE queues. Keep sync.
DMA_ENG = os.environ.get("KERNEL_DMA_ENGINE", "sync")

# (tile, col_start, col_len) exp2 pieces in processing order.
EXP2_PIECES = []
for _t in range(TILES):
    _c = 0
    for _w in ACT_PIECES[_t]:
        EXP2_PIECES.append((_t, _c, _w))
        _c += _w
# DVE squares: one per exp2 piece, clipped to [0, E4_START) on the last tile.
DVE_OPS = []
for (_t, _c, _w) in EXP2_PIECES:
    if _t == TILES - 1:
        _end = min(_c + _w, E4_START)
        if _end <= _c:
            continue
        DVE_OPS.append((_t, _c, _end - _c))
    else:
        DVE_OPS.append((_t, _c, _w))

N_EXP2 = len(EXP2_PIECES)
N_DVE = len(DVE_OPS)
# rs columns: [0, N_EXP2) exp2 rowsums; N_EXP2 = e4 rowsum; then DVE rowsums.
RS_E4 = N_EXP2
NRS = N_EXP2 + 1 + N_DVE

_NC = None
LAST_RESULTS = None  # BassKernelResults of the most recent run (for test harness)


def _build_bass():
    in_dt = mybir.dt.bfloat16
    edt = mybir.dt.bfloat16
    M = B // P  # 32 column blocks

    nc = bacc.Bacc(None)
    slab = nc.declare_dram_parameter("slab", [ROWS_PER_CORE, B], in_dt, isOutput=False)
    rowsums = nc.declare_dram_parameter(
        "rowsums", [P, NRS], mybir.dt.float32, isOutput=True
    )
    # Chunk-stationary layout: [128, 64] (E cols 0:32, E2 cols 32:64), where
    # colsum[m*128 + j] = out[j, m].
    colsums = nc.declare_dram_parameter(
        "colsums", [P, 2 * M], mybir.dt.float32, isOutput=True
    )

    with tile.TileContext(nc) as tc:
        with (
            tc.tile_pool(name="singles", bufs=1) as singles,
            tc.tile_pool(name="psum", bufs=1, space="PSUM") as psum_pool,
        ):
            dmaq = getattr(nc, DMA_ENG)
            ones = nc.const_aps.tensor(1.0, (P, 1), mybir.dt.bfloat16)
            ltile = [
                singles.tile([P, B], in_dt, name=f"ltile{t}") for t in range(TILES)
            ]
            etile = [
                singles.tile([P, B], edt, name=f"etile{t}") for t in range(TILES)
            ]
            e2tile = [
                singles.tile([P, B], edt, name=f"e2tile{t}") for t in range(TILES)
            ]
            # Separate accum tiles per engine so the accumulator-read of one
            # engine never false-depends on the other's accum writes.
            rsA = singles.tile([P, N_EXP2 + 1], mybir.dt.float32)
            rsD = singles.tile([P, N_DVE], mybir.dt.float32)
            cstile = singles.tile([P, 2 * M], mybir.dt.float32)
            psE = psum_pool.tile([P, M], mybir.dt.float32)
            psE2 = psum_pool.tile([P, M], mybir.dt.float32)

            # --- input DMAs, in consumption order ---
            if DMA_SPLIT_LATER:
                for (t, c, w) in EXP2_PIECES:
                    dmaq.dma_start(
                        out=ltile[t][:, c : c + w],
                        in_=slab[t * P : (t + 1) * P, c : c + w],
                    )
            else:
                # tile 0 per piece (early ACT start), tiles 1..3 whole.
                c = 0
                for w in ACT_PIECES[0]:
                    dmaq.dma_start(
                        out=ltile[0][:, c : c + w], in_=slab[0:P, c : c + w]
                    )
                    c += w
                for t in range(1, TILES):
                    dmaq.dma_start(
                        out=ltile[t][:, :], in_=slab[t * P : (t + 1) * P, :]
                    )

            def colsum_blocks(ps, src, t, c, w, first, last):
                # PSUM start_tensor_calc zeroes the whole 2KB (partition, bank)
                # zero-region lazily: only the FIRST matmul touching each psum
                # tensor carries start=True; the LAST carries stop=True.
                for m in range(c // P, (c + w) // P):
                    lsl = slice(m * P, (m + 1) * P)
                    nc.tensor.matmul(
                        ps[:, m : m + 1],
                        src[:, lsl],
                        ones,
                        start=first and m == c // P,
                        stop=last and m == (c + w) // P - 1,
                        skip_group_check=True,
                    )

            # --- main pipeline: exp2 (ACT) -> square (DVE) -> colsums (PE) ---
            dve_iter = iter(enumerate(DVE_OPS))
            next_dve = next(dve_iter, None)
            for i, (t, c, w) in enumerate(EXP2_PIECES):
                nc.scalar.activation(
                    out=etile[t][:, c : c + w],
                    in_=ltile[t][:, c : c + w],
                    func=mybir.ActivationFunctionType.Exp,
                    scale=2.0,
                    accum_out=rsA[:, i : i + 1],
                )
                colsum_blocks(psE, etile[t], t, c, w, first=i == 0, last=i == N_EXP2 - 1)
                # matching DVE square (may be clipped on the last tile)
                if next_dve is not None:
                    j, (dt_, dc, dw) = next_dve
                    if dt_ == t and dc == c:
                        nc.vector.scalar_tensor_tensor(
                            out=e2tile[t][:, dc : dc + dw],
                            in0=etile[t][:, dc : dc + dw],
                            scalar=1.0,
                            in1=etile[t][:, dc : dc + dw],
                            op0=mybir.AluOpType.mult,
                            op1=mybir.AluOpType.mult,
                            accum_out=rsD[:, j : j + 1],
                        )
                        colsum_blocks(
                            psE2, e2tile[t], t, dc, dw, first=j == 0, last=False
                        )
                        next_dve = next(dve_iter, None)

            # --- E4 tail on ACT: E2 = exp(4x) straight from the input ---
            t = TILES - 1
            nc.scalar.activation(
                out=e2tile[t][:, E4_START:],
                in_=ltile[t][:, E4_START:],
                func=mybir.ActivationFunctionType.Exp,
                scale=4.0,
                accum_out=rsA[:, RS_E4 : RS_E4 + 1],
            )
            colsum_blocks(psE2, e2tile[t], t, E4_START, E4_W, first=False, last=True)

            # Tail: spread the output DMA issues across the per-engine HWDGE
            # queues so they overlap instead of serializing on sync.
            # vector: psE eviction + its own rowsums; scalar: psE2 eviction
            # (after the e4 PSUM stop) + its rowsums; sync: colsums (waits on
            # both evictions via sems, issue overlaps them).
            nc.vector.tensor_copy(cstile[:, 0:M], psE)
            dmaq.dma_start(out=rowsums[:, N_EXP2 + 1 :], in_=rsD)
            nc.vector.tensor_copy(cstile[:, M : 2 * M], psE2)
            nc.scalar.dma_start(out=rowsums[:, 0 : N_EXP2 + 1], in_=rsA)
            dmaq.dma_start(out=colsums[:, :], in_=cstile)
    # Bacc defers register allocation and sync-wait splitting to finalize();
    # run_bass_via_pjrt does not call it, so do it here.
    nc.finalize()
    return nc


def _get_nc():
    global _NC
    if _NC is None:
        _NC = _build_bass()
    return _NC


def kernel(logits: np.ndarray) -> np.ndarray:
    global LAST_RESULTS
    logits = np.ascontiguousarray(np.asarray(logits, dtype=np.float32))
    assert logits.shape == (B, B)

    nc = _get_nc()
    cast = lambda a: np.ascontiguousarray(a.astype(ml_dtypes.bfloat16))
    in_maps = [
        {"slab": cast(logits[k * ROWS_PER_CORE : (k + 1) * ROWS_PER_CORE, :])}
        for k in range(N_CORES)
    ]
    res = run_bass_kernel_spmd(
        nc,
        in_maps,
        core_ids=list(range(N_CORES)),
        trace=bool(int(os.environ.get("KERNEL_TRACE", "0"))),
    )
    LAST_RESULTS = res

    M = B // P
    rowsum_E = np.empty(B, dtype=np.float64)
    rowsum_E2 = np.empty(B, dtype=np.float64)
    colsum_E = np.zeros(B, dtype=np.float64)
    colsum_E2 = np.zeros(B, dtype=np.float64)
    for k in range(N_CORES):
        r = res.results[k]
        rsv = r["rowsums"].astype(np.float64)
        sl = slice(k * ROWS_PER_CORE, (k + 1) * ROWS_PER_CORE)
        rsE = np.zeros((P, TILES))
        rsE2 = np.zeros((P, TILES))
        for i, (t, _, _) in enumerate(EXP2_PIECES):
            rsE[:, t] += rsv[:, i]
        if E4_W > 0:
            rsE2[:, TILES - 1] += rsv[:, RS_E4]
        for j, (t, _, _) in enumerate(DVE_OPS):
            rsE2[:, t] += rsv[:, N_EXP2 + 1 + j]
        rowsum_E[sl] = rsE.T.reshape(-1)
        rowsum_E2[sl] = rsE2.T.reshape(-1)
        cssum = r["colsums"].astype(np.float64)
        colsum_E += cssum[:, :M].T.reshape(-1)
        colsum_E2 += cssum[:, M:].T.reshape(-1)

    d = np.diagonal(logits)
    pos = np.exp(d.astype(np.float64) / TEMPERATURE)
    # The device sums contain exp of the bf16-rounded diagonal; subtract
    # exactly what the device added.
    dD = d.astype(ml_dtypes.bfloat16).astype(np.float64)
    posD = np.exp(dD / TEMPERATURE)
    N = 2 * B - 2
    S1 = rowsum_E + colsum_E - 2.0 * posD
    S2 = rowsum_E2 + colsum_E2 - 2.0 * posD * posD
    reweight = S2 * N / S1
    Ng = (-TAU_PLUS * N * pos + reweight) / (1.0 - TAU_PLUS)
    Ng = np.maximum(Ng, N * np.exp(-1.0 / TEMPERATURE))
    loss = -np.log(pos / (pos + Ng + EPS))
    return np.float32(loss.mean())
